# revision 21
# baseline (speedup 1.0000x reference)
"""FastGTLayer GNN message passing on 8 Trainium2 NeuronCores.

Strategy (destination-sharded, gather + selection-matmul scatter), v3:
- Host: softmax(weight) -> per-edge per-channel weights w_c = filt[c,t]*ev[t,e].
  Edges sharded by destination row range (6250 rows/core), sorted by row,
  grouped into 32-row "blocks"; each block padded to a per-block tile quota
  (max over cores) of 128-edge tiles. H pre-transposed to [N, (c,d)] bf16.
- Device (SPMD, one program on 8 cores): for each chunk of 8 blocks, the
  bf16 H rows (both channels interleaved, 256B/edge) are fetched by THREE
  concurrent dma_gather instructions on rotating SWDGE queues (concurrency
  across queues engages more DMA engines: measured ~2.5x gather speedup vs
  one instruction; int16 idx with biased base in_=H_pre[17232:],
  idx=col-17232). DVE builds, chunk-wide, a one-hot sel_eq[e,r]=(row==r)
  and weighted sel01[e,(c,r)] = w_c[e]*sel_eq in bf16; PE scatter-adds with
  ONE bf16 matmul per 128-edge tile: psum[128(c,f), 64(c',r)] += g^T @
  sel01 where the diagonal (c==c') halves hold the per-channel results
  (one bank per 8-block chunk); ACT evicts the two diagonal halves to
  SBUF; HWDGE writes [128,(c,d)] x rows to HBM. Edge slot order is kept
  random (sorting by col clusters HBM banks and measured ~2x slower).
- Host: transpose per-core [128, rows] outputs into [C, N, D].
"""
import os
import sys
if "/opt/trn_rl_repo" not in sys.path:
    sys.path.insert(0, "/opt/trn_rl_repo")
# recover cleanly if a previous process left the cores wedged (no-op on a
# healthy device; only read at NRT init, so harmless if jax already started)
os.environ.setdefault("NEURON_RT_RESET_CORES", "1")

import numpy as np
import ml_dtypes

C, T, N, E, D = 2, 4, 50000, 400000, 64
M = T * E
NCORES = 8
RPC = N // NCORES          # 6250 destination rows per core
R = 32                     # rows per block (psum window)
NBLOCKS = (RPC + R - 1) // R   # 196 (last block partially used)
BIAS = N - 32768           # 17232; idx = col - BIAS in [-17232, 32767]
PADCOL = 40000             # pad slots gather this row (positive idx), weight 0
CHUNK_BLOCKS = 8           # blocks per chunk (psum group / gather batch)
SPLIT_GATHER = 3           # concurrent gather instructions per chunk

_prog_cache = {}
_SLOT_SORT = False  # sorting hurts: clustered HBM rows serialize on banks


def _build_program(quotas, tt, skip_gather=False, skip_compute=False,
                   nqueues=4, chunk_blocks=None, gbufs=2, selbufs=2, pbufs=3,
                   scratch=16384, repeat=1, single_packet=False,
                   skip_sel=False, skip_mm=False, skip_evict=False,
                   split_gather=1, dup_mm=1, dup_sel=1):
    """Build the SPMD Bass program for per-block tile quotas `quotas` (len
    NBLOCKS, sum tt). Returns compiled Bacc instance."""
    from concourse import bacc, mybir
    import concourse.tile as tile
    from concourse.bass import AP

    nc = bacc.Bacc("TRN2", num_swdge_queues=nqueues, dynamic_dma_scratch_size=scratch)
    hpre = nc.dram_tensor("hpre", [N, 2 * D], mybir.dt.bfloat16, kind="ExternalInput")
    idx = nc.dram_tensor("idx", [128, tt * 8], mybir.dt.int16, kind="ExternalInput")
    rowl = nc.dram_tensor("rowl", [128, tt], mybir.dt.bfloat16, kind="ExternalInput")
    w0 = nc.dram_tensor("w0", [128, tt], mybir.dt.bfloat16, kind="ExternalInput")
    w1 = nc.dram_tensor("w1", [128, tt], mybir.dt.bfloat16, kind="ExternalInput")
    iota = nc.dram_tensor("iota", [128, R], mybir.dt.bfloat16, kind="ExternalInput")
    out_local = nc.dram_tensor("out_local", [128, NBLOCKS * R], mybir.dt.float32,
                               kind="ExternalOutput")

    cb_n = chunk_blocks or CHUNK_BLOCKS
    nchunks = (NBLOCKS + cb_n - 1) // cb_n
    tile_base = np.concatenate([[0], np.cumsum(quotas)]).astype(int)

    with tile.TileContext(nc) as tc:
        with tc.tile_pool(name="meta", bufs=1) as mp, \
             tc.tile_pool(name="gp", bufs=gbufs) as gp, \
             tc.tile_pool(name="selp", bufs=selbufs) as selp, \
             tc.tile_pool(name="stp", bufs=2) as stp, \
             tc.tile_pool(name="pp", bufs=pbufs, space="PSUM") as pp:
            idx_t = mp.tile([128, tt * 8], mybir.dt.int16)
            rowl_t = mp.tile([128, tt], mybir.dt.bfloat16)
            w0_t = mp.tile([128, tt], mybir.dt.bfloat16)
            w1_t = mp.tile([128, tt], mybir.dt.bfloat16)
            iota_t = mp.tile([128, R], mybir.dt.bfloat16)

            nc.sync.dma_start(out=idx_t[:], in_=idx[:])
            nc.sync.dma_start(out=rowl_t[:], in_=rowl[:])
            nc.sync.dma_start(out=w0_t[:], in_=w0[:])
            nc.sync.dma_start(out=w1_t[:], in_=w1[:])
            nc.sync.dma_start(out=iota_t[:], in_=iota[:])

            iota_ap = iota_t[:]

            for rep in range(repeat):
              for c in range(nchunks):
                b0 = c * cb_n
                b1 = min(b0 + cb_n, NBLOCKS)
                nb = b1 - b0
                tb0, tb1 = tile_base[b0], tile_base[b1]
                ct = int(tb1 - tb0)          # tiles in this chunk
                nidx = ct * 128

                g_t = gp.tile([128, ct, 2 * D], mybir.dt.bfloat16, tag="g")
                if not skip_gather:
                    sg = split_gather
                    splits = [(i * ct // sg, (i + 1) * ct // sg)
                              for i in range(sg)]
                    for i, (s0, s1) in enumerate(splits):
                        nc.gpsimd.dma_gather(
                            g_t[:, s0:s1, :],
                            hpre[BIAS:, :],
                            idx_t[:, (tb0 + s0) * 8: (tb0 + s1) * 8],
                            (s1 - s0) * 128,
                            (s1 - s0) * 128,
                            2 * D,
                            queue_num=((c * sg + i) % nqueues),
                            single_packet=single_packet,
                        )

                stage = stp.tile([128, nb * R], mybir.dt.float32, tag="st")
                if not skip_compute:
                    # chunk-wide selection build (3 DVE ops for the whole chunk)
                    sel01 = selp.tile([128, ct, 2 * R], mybir.dt.bfloat16, tag="s01")
                    if skip_sel:
                        nc.vector.memset(sel01[:], 0.0)
                    else:
                        iota_c = AP(iota_ap.tensor, iota_ap.offset,
                                    [iota_ap.ap[0], [0, ct], iota_ap.ap[1]])
                        sel_eq = selp.tile([128, ct, R], mybir.dt.bfloat16,
                                           tag="se")
                        for _ in range(dup_sel):
                            nc.vector.tensor_tensor(
                                out=sel_eq[:],
                                in0=rowl_t[:, tb0:tb1].to_broadcast([128, ct, R]),
                                in1=iota_c, op=mybir.AluOpType.is_equal)
                            nc.vector.tensor_tensor(
                                out=sel01[:, :, 0:R], in0=sel_eq[:],
                                in1=w0_t[:, tb0:tb1].to_broadcast([128, ct, R]),
                                op=mybir.AluOpType.mult)
                            nc.vector.tensor_tensor(
                                out=sel01[:, :, R:2 * R], in0=sel_eq[:],
                                in1=w1_t[:, tb0:tb1].to_broadcast([128, ct, R]),
                                op=mybir.AluOpType.mult)

                    if not skip_mm:
                        ps = pp.tile([128, nb, 2 * R], mybir.dt.float32,
                                     space="PSUM", tag="ps")
                        for b in range(b0, b1):
                            kb = int(quotas[b])
                            t0 = int(tile_base[b])      # global tile index
                            lt0 = t0 - tb0              # tile index within chunk
                            for dmm in range(dup_mm):
                                for k in range(kb):
                                    nc.tensor.matmul(
                                        out=ps[:, b - b0, :],
                                        lhsT=g_t[:, lt0 + k, :],
                                        rhs=sel01[:, lt0 + k, :],
                                        start=(dmm == 0 and k == 0),
                                        stop=(dmm == dup_mm - 1 and k == kb - 1))
                    if skip_mm or skip_evict:
                        nc.vector.memset(stage[:], 0.0)
                    else:
                        # evict diagonal halves: ch0 x sel0 cols, ch1 x sel1
                        nc.scalar.copy(out=stage[0:64, :],
                                       in_=ps[0:64, :, 0:R])
                        nc.scalar.copy(out=stage[64:128, :],
                                       in_=ps[64:128, :, R:2 * R])
                else:
                    nc.vector.memset(stage[:], 0.0)
                nc.sync.dma_start(out=out_local[:, b0 * R: b1 * R], in_=stage[:])

    nc.compile()
    return nc


def _prepare(H_, edge_index, edge_values, weight):
    """Host-side preprocessing. Returns (quotas, tt, in_maps)."""
    H_ = np.asarray(H_, dtype=np.float32)
    edge_index = np.asarray(edge_index)
    edge_values = np.asarray(edge_values, dtype=np.float32)
    weight = np.asarray(weight, dtype=np.float64)

    # softmax over edge types per channel
    wexp = np.exp(weight - weight.max(axis=1, keepdims=True))
    filt = (wexp / wexp.sum(axis=1, keepdims=True)).astype(np.float32)  # [C,T]

    row = np.ascontiguousarray(edge_index[:, 0, :]).reshape(-1).astype(np.int64)
    col = np.ascontiguousarray(edge_index[:, 1, :]).reshape(-1).astype(np.int64)
    ev = edge_values.reshape(-1)
    tt_of_edge = np.repeat(np.arange(T), E)
    wc = filt[:, tt_of_edge] * ev[None, :]      # [C, M]

    H_pre = np.ascontiguousarray(
        np.transpose(H_, (1, 0, 2)).reshape(N, C * D).astype(ml_dtypes.bfloat16))

    core = row // RPC
    row_local = row - core * RPC
    block = row_local // R                       # [0, NBLOCKS)
    # global sort groups edges by (core, block) since both are row-range based
    perm = np.argsort(row, kind="stable")
    core_s = core[perm]
    block_s = block[perm]
    col_s = col[perm]
    rl_s = (row_local[perm] % R).astype(np.float32)
    w0_s = wc[0][perm]
    w1_s = wc[1][perm]

    cb = core_s * NBLOCKS + block_s
    counts = np.bincount(cb, minlength=NCORES * NBLOCKS).reshape(NCORES, NBLOCKS)
    quotas = np.ceil(counts.max(axis=0) / 128).astype(np.int64)
    quotas = np.maximum(quotas, 1)
    tt = int(quotas.sum())
    tile_base = np.concatenate([[0], np.cumsum(quotas)]).astype(np.int64)

    # slot index for each sorted edge: block b of core k occupies slots
    # [tile_base[b]*128, +counts[k,b]) in core k's slot space
    group_starts = np.concatenate([[0], np.cumsum(counts.reshape(-1))])[:-1]
    within = np.arange(M) - group_starts[cb]     # position within group
    slot = tile_base[block_s] * 128 + within     # per-core slot index

    nslots = tt * 128
    idx16 = np.full((NCORES, nslots), PADCOL - BIAS, dtype=np.int16)
    rowl_a = np.zeros((NCORES, nslots), dtype=np.float32)
    w0_a = np.zeros((NCORES, nslots), dtype=np.float32)
    w1_a = np.zeros((NCORES, nslots), dtype=np.float32)

    idx16[core_s, slot] = (col_s - BIAS).astype(np.int16)
    rowl_a[core_s, slot] = rl_s
    w0_a[core_s, slot] = w0_s.astype(np.float32)
    w1_a[core_s, slot] = w1_s.astype(np.float32)

    if _SLOT_SORT:
        # sort slots by source col within each block (DRAM row locality for
        # the gather); permutes idx/rowl/w jointly so results are unchanged
        block_of_slot = np.repeat(np.arange(NBLOCKS, dtype=np.int64),
                                  quotas * 128)
        for k in range(NCORES):
            key = block_of_slot * 70000 + (idx16[k].astype(np.int64) + BIAS)
            order = np.argsort(key, kind="stable")
            idx16[k] = idx16[k][order]
            rowl_a[k] = rowl_a[k][order]
            w0_a[k] = w0_a[k][order]
            w1_a[k] = w1_a[k][order]

    # ensure the LAST slot of every gather instruction has idx >= 0
    # (dma_gather trims a trailing negative run); swap within the final
    # tile if needed. Instruction boundaries: each chunk's tile range is
    # split into SPLIT_GATHER concurrent gathers.
    nchunks = (NBLOCKS + CHUNK_BLOCKS - 1) // CHUNK_BLOCKS
    for cidx in range(nchunks):
        b0 = cidx * CHUNK_BLOCKS
        b1 = min(b0 + CHUNK_BLOCKS, NBLOCKS)
        tb0, tb1 = int(tile_base[b0]), int(tile_base[b1])
        ct = tb1 - tb0
        for i in range(SPLIT_GATHER):
            s1 = (i + 1) * ct // SPLIT_GATHER
            s0 = i * ct // SPLIT_GATHER
            if s1 == s0:
                continue
            end = (tb0 + s1) * 128       # one past instruction's last slot
            for k in range(NCORES):
                if idx16[k, end - 1] < 0:
                    tile_lo = end - 128
                    cand = np.nonzero(idx16[k, tile_lo:end - 1] >= 0)[0]
                    assert cand.size > 0, "entire tile has negative idx"
                    j = tile_lo + cand[-1]
                    for arr in (idx16, rowl_a, w0_a, w1_a):
                        arr[k, j], arr[k, end - 1] = arr[k, end - 1], arr[k, j]

    iota_np = np.tile(np.arange(R, dtype=np.float32), (128, 1)).astype(
        ml_dtypes.bfloat16)
    bf = ml_dtypes.bfloat16
    in_maps = []
    for k in range(NCORES):
        in_maps.append({
            "hpre": H_pre,
            # idx position q -> partition q%16, free q//16; replicate x8
            "idx": np.ascontiguousarray(
                np.tile(idx16[k].reshape(nslots // 16, 16).T, (8, 1))),
            "rowl": np.ascontiguousarray(rowl_a[k].reshape(tt, 128).T.astype(bf)),
            "w0": np.ascontiguousarray(w0_a[k].reshape(tt, 128).T.astype(bf)),
            "w1": np.ascontiguousarray(w1_a[k].reshape(tt, 128).T.astype(bf)),
            "iota": iota_np,
        })
    return tuple(quotas.tolist()), tt, in_maps


def _make_runner(nc):
    """Build and cache a jitted shard_map executor for the compiled program."""
    import jax
    from jax.sharding import Mesh, PartitionSpec, NamedSharding
    from jax.experimental.shard_map import shard_map
    from concourse import mybir
    from concourse.bass2jax import (_bass_exec_p, partition_id_tensor,
                                    install_neuronx_cc_hook)

    install_neuronx_cc_hook()
    partition_name = nc.partition_id_tensor.name if nc.partition_id_tensor else None
    in_names, out_names, out_avals = [], [], []
    for alloc in nc.m.functions[0].allocations:
        if not isinstance(alloc, mybir.MemoryLocationSet):
            continue
        name = alloc.memorylocations[0].name
        if alloc.kind == "ExternalInput":
            if name != partition_name:
                in_names.append(name)
        elif alloc.kind == "ExternalOutput":
            out_names.append(name)
            out_avals.append(jax.core.ShapedArray(
                tuple(alloc.tensor_shape), mybir.dt.np(alloc.dtype)))
    n_params = len(in_names)
    all_in = in_names + out_names + ([partition_name] if partition_name else [])

    def _body(*args):
        operands = list(args)
        if partition_name is not None:
            operands.append(partition_id_tensor())
        return tuple(_bass_exec_p.bind(
            *operands, out_avals=tuple(out_avals), in_names=tuple(all_in),
            out_names=tuple(out_names), lowering_input_output_aliases=(),
            sim_require_finite=True, sim_require_nnan=True, nc=nc))

    devices = jax.devices()[:NCORES]
    mesh = Mesh(np.asarray(devices), ("core",))
    spec = PartitionSpec("core")
    f = jax.jit(shard_map(_body, mesh=mesh,
                          in_specs=(spec,) * (n_params + len(out_names)),
                          out_specs=(spec,), check_rep=False))
    sharding = NamedSharding(mesh, spec)
    zeros = [np.zeros((av.shape[0] * NCORES,) + av.shape[1:], av.dtype)
             for av in out_avals]
    return {"f": f, "in_names": in_names, "out_names": out_names,
            "sharding": sharding, "zeros": zeros}


def kernel(H_, edge_index, edge_values, weight, num_nodes):
    import jax

    quotas, tt, in_maps = _prepare(H_, edge_index, edge_values, weight)
    key = quotas
    if key not in _prog_cache:
        nc = _build_program(np.array(quotas), tt, split_gather=SPLIT_GATHER,
                            gbufs=3, selbufs=3)
        _prog_cache[key] = _make_runner(nc)
    rn = _prog_cache[key]

    args = []
    for name in rn["in_names"]:
        glob = np.concatenate([m[name] for m in in_maps], axis=0)
        args.append(jax.device_put(glob, rn["sharding"]))
    for z in rn["zeros"]:
        args.append(jax.device_put(z, rn["sharding"]))
    outs = rn["f"](*args)
    res = np.asarray(outs[rn["out_names"].index("out_local")])  # [8*128, NBLOCKS*R]

    out = np.empty((C, N, D), dtype=np.float32)
    for k in range(NCORES):
        ol = res[k * 128:(k + 1) * 128]
        out[0, k * RPC:(k + 1) * RPC, :] = ol[0:D, :RPC].T
        out[1, k * RPC:(k + 1) * RPC, :] = ol[D:2 * D, :RPC].T
    return out


# revision 23
# speedup vs baseline: 1.1093x; 1.1093x over previous
"""FastGTLayer GNN message passing on 8 Trainium2 NeuronCores.

Strategy (destination-sharded, gather + selection-matmul scatter), v3:
- Host: softmax(weight) -> per-edge per-channel weights w_c = filt[c,t]*ev[t,e].
  Edges sharded by destination row range (6250 rows/core), sorted by row,
  grouped into 32-row "blocks"; each block padded to a per-block tile quota
  (max over cores) of 128-edge tiles. H pre-transposed to [N, (c,d)] bf16.
- Device (SPMD, one program on 8 cores): for each chunk of 8 blocks, the
  bf16 H rows (both channels interleaved, 256B/edge) are fetched by THREE
  concurrent dma_gather instructions on rotating SWDGE queues (concurrency
  across queues engages more DMA engines: measured ~2.5x gather speedup vs
  one instruction; int16 idx with biased base in_=H_pre[17232:],
  idx=col-17232). DVE builds, chunk-wide, a one-hot sel_eq[e,r]=(row==r)
  and weighted sel01[e,(c,r)] = w_c[e]*sel_eq in bf16; PE scatter-adds with
  ONE bf16 matmul per 128-edge tile: psum[128(c,f), 64(c',r)] += g^T @
  sel01 where the diagonal (c==c') halves hold the per-channel results
  (one bank per 8-block chunk); ACT evicts the two diagonal halves to
  SBUF; HWDGE writes [128,(c,d)] x rows to HBM. Edge slot order is kept
  random (sorting by col clusters HBM banks and measured ~2x slower).
- Host: transpose per-core [128, rows] outputs into [C, N, D].
"""
import os
import sys
if "/opt/trn_rl_repo" not in sys.path:
    sys.path.insert(0, "/opt/trn_rl_repo")
# recover cleanly if a previous process left the cores wedged (no-op on a
# healthy device; only read at NRT init, so harmless if jax already started)
os.environ.setdefault("NEURON_RT_RESET_CORES", "1")

import numpy as np
import ml_dtypes

C, T, N, E, D = 2, 4, 50000, 400000, 64
M = T * E
NCORES = 8
RPC = N // NCORES          # 6250 destination rows per core
R = 32                     # rows per block (psum window)
NBLOCKS = (RPC + R - 1) // R   # 196 (last block partially used)
BIAS = N - 32768           # 17232; idx = col - BIAS in [-17232, 32767]
PADCOL = 40000             # pad slots gather this row (positive idx), weight 0
CHUNK_BLOCKS = 12          # blocks per chunk (psum group / gather batch)
SPLIT_GATHER = 3           # concurrent gather instructions per chunk

_prog_cache = {}
_SLOT_SORT = False  # sorting hurts: clustered HBM rows serialize on banks


def _build_program(quotas, tt, skip_gather=False, skip_compute=False,
                   nqueues=4, chunk_blocks=None, gbufs=2, selbufs=2, pbufs=3,
                   scratch=16384, repeat=1, single_packet=False,
                   skip_sel=False, skip_mm=False, skip_evict=False,
                   split_gather=1, dup_mm=1, dup_sel=1):
    """Build the SPMD Bass program for per-block tile quotas `quotas` (len
    NBLOCKS, sum tt). Returns compiled Bacc instance."""
    from concourse import bacc, mybir
    import concourse.tile as tile
    from concourse.bass import AP

    nc = bacc.Bacc("TRN2", num_swdge_queues=nqueues, dynamic_dma_scratch_size=scratch)
    hpre = nc.dram_tensor("hpre", [N, 2 * D], mybir.dt.bfloat16, kind="ExternalInput")
    idx = nc.dram_tensor("idx", [128, tt * 8], mybir.dt.int16, kind="ExternalInput")
    rowl = nc.dram_tensor("rowl", [128, tt], mybir.dt.bfloat16, kind="ExternalInput")
    w0 = nc.dram_tensor("w0", [128, tt], mybir.dt.bfloat16, kind="ExternalInput")
    w1 = nc.dram_tensor("w1", [128, tt], mybir.dt.bfloat16, kind="ExternalInput")
    iota = nc.dram_tensor("iota", [128, R], mybir.dt.bfloat16, kind="ExternalInput")
    out_local = nc.dram_tensor("out_local", [128, NBLOCKS * R], mybir.dt.float32,
                               kind="ExternalOutput")

    cb_n = chunk_blocks or CHUNK_BLOCKS
    nchunks = (NBLOCKS + cb_n - 1) // cb_n
    tile_base = np.concatenate([[0], np.cumsum(quotas)]).astype(int)

    with tile.TileContext(nc) as tc:
        with tc.tile_pool(name="meta", bufs=1) as mp, \
             tc.tile_pool(name="gp", bufs=gbufs) as gp, \
             tc.tile_pool(name="selp", bufs=selbufs) as selp, \
             tc.tile_pool(name="stp", bufs=2) as stp, \
             tc.tile_pool(name="pp", bufs=pbufs, space="PSUM") as pp:
            idx_t = mp.tile([128, tt * 8], mybir.dt.int16)
            rowl_t = mp.tile([128, tt], mybir.dt.bfloat16)
            w0_t = mp.tile([128, tt], mybir.dt.bfloat16)
            w1_t = mp.tile([128, tt], mybir.dt.bfloat16)
            iota_t = mp.tile([128, R], mybir.dt.bfloat16)

            nc.sync.dma_start(out=idx_t[:], in_=idx[:])
            nc.sync.dma_start(out=rowl_t[:], in_=rowl[:])
            nc.sync.dma_start(out=w0_t[:], in_=w0[:])
            nc.sync.dma_start(out=w1_t[:], in_=w1[:])
            nc.sync.dma_start(out=iota_t[:], in_=iota[:])

            iota_ap = iota_t[:]

            for rep in range(repeat):
              for c in range(nchunks):
                b0 = c * cb_n
                b1 = min(b0 + cb_n, NBLOCKS)
                nb = b1 - b0
                tb0, tb1 = tile_base[b0], tile_base[b1]
                ct = int(tb1 - tb0)          # tiles in this chunk
                nidx = ct * 128

                g_t = gp.tile([128, ct, 2 * D], mybir.dt.bfloat16, tag="g")
                if not skip_gather:
                    sg = split_gather
                    splits = [(i * ct // sg, (i + 1) * ct // sg)
                              for i in range(sg)]
                    for i, (s0, s1) in enumerate(splits):
                        nc.gpsimd.dma_gather(
                            g_t[:, s0:s1, :],
                            hpre[BIAS:, :],
                            idx_t[:, (tb0 + s0) * 8: (tb0 + s1) * 8],
                            (s1 - s0) * 128,
                            (s1 - s0) * 128,
                            2 * D,
                            queue_num=((c * sg + i) % nqueues),
                            single_packet=single_packet,
                        )

                stage = stp.tile([128, nb * R], mybir.dt.float32, tag="st")
                if not skip_compute:
                    # chunk-wide selection build (3 DVE ops for the whole chunk)
                    sel01 = selp.tile([128, ct, 2 * R], mybir.dt.bfloat16, tag="s01")
                    if skip_sel:
                        nc.vector.memset(sel01[:], 0.0)
                    else:
                        iota_c = AP(iota_ap.tensor, iota_ap.offset,
                                    [iota_ap.ap[0], [0, ct], iota_ap.ap[1]])
                        sel_eq = selp.tile([128, ct, R], mybir.dt.bfloat16,
                                           tag="se")
                        for _ in range(dup_sel):
                            nc.vector.tensor_tensor(
                                out=sel_eq[:],
                                in0=rowl_t[:, tb0:tb1].to_broadcast([128, ct, R]),
                                in1=iota_c, op=mybir.AluOpType.is_equal)
                            nc.vector.tensor_tensor(
                                out=sel01[:, :, 0:R], in0=sel_eq[:],
                                in1=w0_t[:, tb0:tb1].to_broadcast([128, ct, R]),
                                op=mybir.AluOpType.mult)
                            nc.vector.tensor_tensor(
                                out=sel01[:, :, R:2 * R], in0=sel_eq[:],
                                in1=w1_t[:, tb0:tb1].to_broadcast([128, ct, R]),
                                op=mybir.AluOpType.mult)

                    if not skip_mm:
                        ps = pp.tile([128, nb, 2 * R], mybir.dt.float32,
                                     space="PSUM", tag="ps")
                        for b in range(b0, b1):
                            kb = int(quotas[b])
                            t0 = int(tile_base[b])      # global tile index
                            lt0 = t0 - tb0              # tile index within chunk
                            for dmm in range(dup_mm):
                                for k in range(kb):
                                    nc.tensor.matmul(
                                        out=ps[:, b - b0, :],
                                        lhsT=g_t[:, lt0 + k, :],
                                        rhs=sel01[:, lt0 + k, :],
                                        start=(dmm == 0 and k == 0),
                                        stop=(dmm == dup_mm - 1 and k == kb - 1))
                    if skip_mm or skip_evict:
                        nc.vector.memset(stage[:], 0.0)
                    else:
                        # evict diagonal halves: ch0 x sel0 cols, ch1 x sel1
                        nc.scalar.copy(out=stage[0:64, :],
                                       in_=ps[0:64, :, 0:R])
                        nc.scalar.copy(out=stage[64:128, :],
                                       in_=ps[64:128, :, R:2 * R])
                else:
                    nc.vector.memset(stage[:], 0.0)
                nc.sync.dma_start(out=out_local[:, b0 * R: b1 * R], in_=stage[:])

    nc.compile()
    return nc


def _prepare(H_, edge_index, edge_values, weight):
    """Host-side preprocessing. Returns (quotas, tt, in_maps)."""
    H_ = np.asarray(H_, dtype=np.float32)
    edge_index = np.asarray(edge_index)
    edge_values = np.asarray(edge_values, dtype=np.float32)
    weight = np.asarray(weight, dtype=np.float64)

    # softmax over edge types per channel
    wexp = np.exp(weight - weight.max(axis=1, keepdims=True))
    filt = (wexp / wexp.sum(axis=1, keepdims=True)).astype(np.float32)  # [C,T]

    row = np.ascontiguousarray(edge_index[:, 0, :]).reshape(-1).astype(np.int64)
    col = np.ascontiguousarray(edge_index[:, 1, :]).reshape(-1).astype(np.int64)
    ev = edge_values.reshape(-1)
    tt_of_edge = np.repeat(np.arange(T), E)
    wc = filt[:, tt_of_edge] * ev[None, :]      # [C, M]

    H_pre = np.ascontiguousarray(
        np.transpose(H_, (1, 0, 2)).reshape(N, C * D).astype(ml_dtypes.bfloat16))

    core = row // RPC
    row_local = row - core * RPC
    block = row_local // R                       # [0, NBLOCKS)
    # global sort groups edges by (core, block) since both are row-range based
    perm = np.argsort(row, kind="stable")
    core_s = core[perm]
    block_s = block[perm]
    col_s = col[perm]
    rl_s = (row_local[perm] % R).astype(np.float32)
    w0_s = wc[0][perm]
    w1_s = wc[1][perm]

    cb = core_s * NBLOCKS + block_s
    counts = np.bincount(cb, minlength=NCORES * NBLOCKS).reshape(NCORES, NBLOCKS)
    quotas = np.ceil(counts.max(axis=0) / 128).astype(np.int64)
    quotas = np.maximum(quotas, 1)
    tt = int(quotas.sum())
    tile_base = np.concatenate([[0], np.cumsum(quotas)]).astype(np.int64)

    # slot index for each sorted edge: block b of core k occupies slots
    # [tile_base[b]*128, +counts[k,b]) in core k's slot space
    group_starts = np.concatenate([[0], np.cumsum(counts.reshape(-1))])[:-1]
    within = np.arange(M) - group_starts[cb]     # position within group
    slot = tile_base[block_s] * 128 + within     # per-core slot index

    nslots = tt * 128
    idx16 = np.full((NCORES, nslots), PADCOL - BIAS, dtype=np.int16)
    rowl_a = np.zeros((NCORES, nslots), dtype=np.float32)
    w0_a = np.zeros((NCORES, nslots), dtype=np.float32)
    w1_a = np.zeros((NCORES, nslots), dtype=np.float32)

    idx16[core_s, slot] = (col_s - BIAS).astype(np.int16)
    rowl_a[core_s, slot] = rl_s
    w0_a[core_s, slot] = w0_s.astype(np.float32)
    w1_a[core_s, slot] = w1_s.astype(np.float32)

    if _SLOT_SORT:
        # sort slots by source col within each block (DRAM row locality for
        # the gather); permutes idx/rowl/w jointly so results are unchanged
        block_of_slot = np.repeat(np.arange(NBLOCKS, dtype=np.int64),
                                  quotas * 128)
        for k in range(NCORES):
            key = block_of_slot * 70000 + (idx16[k].astype(np.int64) + BIAS)
            order = np.argsort(key, kind="stable")
            idx16[k] = idx16[k][order]
            rowl_a[k] = rowl_a[k][order]
            w0_a[k] = w0_a[k][order]
            w1_a[k] = w1_a[k][order]

    # ensure the LAST slot of every gather instruction has idx >= 0
    # (dma_gather trims a trailing negative run); swap within the final
    # tile if needed. Instruction boundaries: each chunk's tile range is
    # split into SPLIT_GATHER concurrent gathers.
    nchunks = (NBLOCKS + CHUNK_BLOCKS - 1) // CHUNK_BLOCKS
    for cidx in range(nchunks):
        b0 = cidx * CHUNK_BLOCKS
        b1 = min(b0 + CHUNK_BLOCKS, NBLOCKS)
        tb0, tb1 = int(tile_base[b0]), int(tile_base[b1])
        ct = tb1 - tb0
        for i in range(SPLIT_GATHER):
            s1 = (i + 1) * ct // SPLIT_GATHER
            s0 = i * ct // SPLIT_GATHER
            if s1 == s0:
                continue
            end = (tb0 + s1) * 128       # one past instruction's last slot
            for k in range(NCORES):
                if idx16[k, end - 1] < 0:
                    tile_lo = end - 128
                    cand = np.nonzero(idx16[k, tile_lo:end - 1] >= 0)[0]
                    assert cand.size > 0, "entire tile has negative idx"
                    j = tile_lo + cand[-1]
                    for arr in (idx16, rowl_a, w0_a, w1_a):
                        arr[k, j], arr[k, end - 1] = arr[k, end - 1], arr[k, j]

    iota_np = np.tile(np.arange(R, dtype=np.float32), (128, 1)).astype(
        ml_dtypes.bfloat16)
    bf = ml_dtypes.bfloat16
    in_maps = []
    for k in range(NCORES):
        in_maps.append({
            "hpre": H_pre,
            # idx position q -> partition q%16, free q//16; replicate x8
            "idx": np.ascontiguousarray(
                np.tile(idx16[k].reshape(nslots // 16, 16).T, (8, 1))),
            "rowl": np.ascontiguousarray(rowl_a[k].reshape(tt, 128).T.astype(bf)),
            "w0": np.ascontiguousarray(w0_a[k].reshape(tt, 128).T.astype(bf)),
            "w1": np.ascontiguousarray(w1_a[k].reshape(tt, 128).T.astype(bf)),
            "iota": iota_np,
        })
    return tuple(quotas.tolist()), tt, in_maps


def _make_runner(nc):
    """Build and cache a jitted shard_map executor for the compiled program."""
    import jax
    from jax.sharding import Mesh, PartitionSpec, NamedSharding
    from jax.experimental.shard_map import shard_map
    from concourse import mybir
    from concourse.bass2jax import (_bass_exec_p, partition_id_tensor,
                                    install_neuronx_cc_hook)

    install_neuronx_cc_hook()
    partition_name = nc.partition_id_tensor.name if nc.partition_id_tensor else None
    in_names, out_names, out_avals = [], [], []
    for alloc in nc.m.functions[0].allocations:
        if not isinstance(alloc, mybir.MemoryLocationSet):
            continue
        name = alloc.memorylocations[0].name
        if alloc.kind == "ExternalInput":
            if name != partition_name:
                in_names.append(name)
        elif alloc.kind == "ExternalOutput":
            out_names.append(name)
            out_avals.append(jax.core.ShapedArray(
                tuple(alloc.tensor_shape), mybir.dt.np(alloc.dtype)))
    n_params = len(in_names)
    all_in = in_names + out_names + ([partition_name] if partition_name else [])

    def _body(*args):
        operands = list(args)
        if partition_name is not None:
            operands.append(partition_id_tensor())
        return tuple(_bass_exec_p.bind(
            *operands, out_avals=tuple(out_avals), in_names=tuple(all_in),
            out_names=tuple(out_names), lowering_input_output_aliases=(),
            sim_require_finite=True, sim_require_nnan=True, nc=nc))

    devices = jax.devices()[:NCORES]
    mesh = Mesh(np.asarray(devices), ("core",))
    spec = PartitionSpec("core")
    f = jax.jit(shard_map(_body, mesh=mesh,
                          in_specs=(spec,) * (n_params + len(out_names)),
                          out_specs=(spec,), check_rep=False))
    sharding = NamedSharding(mesh, spec)
    zeros = [np.zeros((av.shape[0] * NCORES,) + av.shape[1:], av.dtype)
             for av in out_avals]
    return {"f": f, "in_names": in_names, "out_names": out_names,
            "sharding": sharding, "zeros": zeros}


def kernel(H_, edge_index, edge_values, weight, num_nodes):
    import jax

    quotas, tt, in_maps = _prepare(H_, edge_index, edge_values, weight)
    key = quotas
    if key not in _prog_cache:
        nc = _build_program(np.array(quotas), tt, split_gather=SPLIT_GATHER,
                            gbufs=3, selbufs=2)
        _prog_cache[key] = _make_runner(nc)
    rn = _prog_cache[key]

    args = []
    for name in rn["in_names"]:
        glob = np.concatenate([m[name] for m in in_maps], axis=0)
        args.append(jax.device_put(glob, rn["sharding"]))
    for z in rn["zeros"]:
        args.append(jax.device_put(z, rn["sharding"]))
    outs = rn["f"](*args)
    res = np.asarray(outs[rn["out_names"].index("out_local")])  # [8*128, NBLOCKS*R]

    out = np.empty((C, N, D), dtype=np.float32)
    for k in range(NCORES):
        ol = res[k * 128:(k + 1) * 128]
        out[0, k * RPC:(k + 1) * RPC, :] = ol[0:D, :RPC].T
        out[1, k * RPC:(k + 1) * RPC, :] = ol[D:2 * D, :RPC].T
    return out
